# revision 38
# baseline (speedup 1.0000x reference)
"""AxialAttention TRN2 kernel.

Sharding: 8 cores = 4 batches x 2 head-groups (4 heads each). Each core:
  phase 1: qkv projection (bf16 matmuls, x stationary) -> s-major qkv [S, 768]
           written to DRAM in BOTH row-major (h,w) and col-major (w,h) pixel
           orders; row-major copy split into 4 quarter tiles. Bias handling:
           k-bias cancels in softmax, v-bias is folded into bout on the host
           (sum_u P_u = 1), so only the q-bias is applied on device, as a
           zero-padded K=1 matmul [bq | 0] closing the bank-aligned q,k
           accumulation region (regions narrower than a PSUM bank with
           mismatched chain ends corrupt the accumulation).
  phase 2: row + col axial attention (bf16): scores via k^T q (d contracted),
           exp without max-subtraction (scores bounded), per-query sums Z via
           a ones-matmul stacked per head pair (psZs), 1/Z on DVE
           (reciprocal_approx_fast; the neuronxcc verifier rejects both the
           divide ALU op and TensorTensor with two PSUM operands), drains
           fused with the normalization multiply. Row branch writes O_sb,
           col branch multiplies into a tmp tile and adds into O_sb on the
           Pool engine (strided dst).
  phase 3: output projection (bf16) of row+col sum; bias on group-0 cores
           only (pre-folded with the v-bias term on host); out dram is bf16,
           host upcasts and sums the two head-group partials.

SBUF pools span all phases (no inter-phase SBUF anti-deps); PSUM pools are
phase-scoped. DMA routing: nc.sync carries ONLY transpose DMAs (XBAR
transpose<->copy interleave corrupts data on this HW), nc.scalar the phase-1
streams / col-v quarters 0-1 / out, nc.gpsimd (SWDGE) the row-v stream and
col-v quarters 2-3.
"""

import numpy as np
import ml_dtypes
from contextlib import ExitStack

import concourse.bass as bass
import concourse.bacc as bacc
import concourse.tile as tile
from concourse import mybir
from concourse.bass_utils import run_bass_kernel_spmd

C = 512          # channels
H = 128          # height
W = 128          # width
S = H * W        # 16384 pixels
NH = 8           # total heads
D = 64           # head dim
NHC = 4          # heads per core
GC = NHC * D     # 256 group channels (q or k or v)
QKV = 3 * GC     # 768 projected channels per core
CT = C // 128    # 4 contraction tiles
ST = S // 128    # 128 spatial tiles
NQ = 4           # qkvB quarter tiles
SCALE = 1.0 / np.sqrt(D)

F32 = mybir.dt.float32
BF16 = mybir.dt.bfloat16
EXP = mybir.ActivationFunctionType.Exp
IDENT = mybir.ActivationFunctionType.Identity
ADD = mybir.AluOpType.add
MULT = mybir.AluOpType.mult
DIV = mybir.AluOpType.divide

_CACHED_NC = None


def build_nc(debug_dump=False, reps=1):
    nc = bacc.Bacc()
    x_in = nc.dram_tensor("x", [C, S], BF16, kind="ExternalInput")
    wqkvT = nc.dram_tensor("wqkvT", [C, QKV], BF16, kind="ExternalInput")
    bq = nc.dram_tensor("bq", [128, QKV], F32, kind="ExternalInput")
    woutT = nc.dram_tensor("woutT", [GC, C], BF16, kind="ExternalInput")
    bout = nc.dram_tensor("bout", [128, CT], F32, kind="ExternalInput")
    out = nc.dram_tensor("out", [C, S], BF16, kind="ExternalOutput")
    if debug_dump:
        dbg_qkv = nc.dram_tensor("dbg_qkv", [S, QKV], BF16, kind="ExternalOutput")
        dbg_O = nc.dram_tensor("dbg_O", [2, 128, S], BF16, kind="ExternalOutput")

    with tile.TileContext(nc) as tc, ExitStack() as ctx:
        persist = ctx.enter_context(tc.tile_pool(name="persist", bufs=1))
        dram = ctx.enter_context(tc.tile_pool(name="dram", bufs=1, space="DRAM"))

        # --- persistent tiles ---
        w_sb = persist.tile([128, CT, QKV], BF16, tag="w_sb")
        nc.scalar.dma_start(
            out=w_sb, in_=wqkvT.ap().rearrange("(t p) o -> p t o", p=128)
        )
        bq_sb = persist.tile([128, QKV], F32, tag="bq_sb")
        nc.scalar.dma_start(out=bq_sb, in_=bq.ap())
        wout_sb = persist.tile([128, 2, C], BF16, tag="wout_sb")
        nc.scalar.dma_start(
            out=wout_sb, in_=woutT.ap().rearrange("(t p) o -> p t o", p=128)
        )
        boutv = persist.tile([128, CT], F32, tag="boutv")
        nc.scalar.dma_start(out=boutv, in_=bout.ap())
        ones_sb = persist.tile([128, 128], BF16, tag="ones_sb")
        nc.vector.memset(ones_sb, 1.0)

        O_sb = [
            persist.tile([128, S], BF16, tag=f"O{i}", name=f"O{i}") for i in range(2)
        ]

        # row-major copy in quarters (pixel order s = h*W + w)
        qkvQ = [
            dram.tile([S // NQ, QKV], BF16, tag=f"qkvQ{i}", name=f"qkvQ{i}")
            for i in range(NQ)
        ]
        qkvB2 = dram.tile([S, 512], BF16)   # q,k only; pixel order s' = w*H + h

        for _rep in range(reps):
            build_body(nc, tc, x_in, w_sb, bq_sb, wout_sb, boutv,
                       ones_sb, O_sb, qkvQ, qkvB2, out)

        if debug_dump:
            for qi in range(NQ):
                nc.scalar.dma_start(
                    out=dbg_qkv[qi * (S // NQ) : (qi + 1) * (S // NQ), :],
                    in_=qkvQ[qi][:],
                )
            for i in range(2):
                nc.scalar.dma_start(out=dbg_O.ap()[i], in_=O_sb[i])

    nc.finalize()
    return nc


def build_body(nc, tc, x_in, w_sb, bq_sb, wout_sb, boutv, ones_sb,
               O_sb, qkvQ, qkvB2, out):
    # SBUF pools span all phases (no SBUF anti-deps between phases);
    # PSUM pools stay phase-scoped (8 banks cannot hold all phases at once).
    sbctx = ExitStack()
    with sbctx:
        xpool = sbctx.enter_context(tc.tile_pool(name="p1x", bufs=4))
        opool = sbctx.enter_context(tc.tile_pool(name="p1o", bufs=3))
        qtpool = sbctx.enter_context(tc.tile_pool(name="a_qt", bufs=4))
        ktpool = sbctx.enter_context(tc.tile_pool(name="a_kt", bufs=4))
        vtpool = sbctx.enter_context(tc.tile_pool(name="a_vt", bufs=2))
        ppool = sbctx.enter_context(tc.tile_pool(name="a_p", bufs=3))
        tmppool = sbctx.enter_context(tc.tile_pool(name="a_tmp", bufs=3))
        rzpool = sbctx.enter_context(tc.tile_pool(name="a_rz", bufs=2))
        fpool = sbctx.enter_context(tc.tile_pool(name="f_o", bufs=3))
        build_phases(nc, tc, x_in, w_sb, bq_sb, wout_sb, boutv,
                     ones_sb, O_sb, qkvQ, qkvB2, out, xpool, opool, qtpool,
                     ktpool, vtpool, ppool, tmppool, rzpool, fpool)


def build_phases(nc, tc, x_in, w_sb, bq_sb, wout_sb, boutv, ones_sb,
                 O_sb, qkvQ, qkvB2, out, xpool, opool, qtpool, ktpool, vtpool,
                 ppool, tmppool, rzpool, fpool):
    # ---------- phase 1: qkv projection (x stationary, s-major out) ----------
    x_r = x_in.ap().rearrange("(t p) s -> p t s", p=128)
    with tc.tile_pool(name="p1ps", bufs=3, space="PSUM") as pspool:
        for sg in range(ST // 4):  # groups of 4 s-tiles (one h-quad)
            if sg % 2 == 0:
                xg = xpool.tile([128, CT, 1024], BF16)
                nc.scalar.dma_start(
                    out=xg, in_=x_r[:, :, sg * 512 : sg * 512 + 1024]
                )
            qt4 = opool.tile([128, 4, QKV], BF16)
            for i in range(4):
                ps = pspool.tile([128, QKV], F32)
                for ct in range(CT):
                    lhsT = xg[
                        :, ct,
                        (sg % 2) * 512 + i * 128 : (sg % 2) * 512 + (i + 1) * 128,
                    ]
                    # two bank-aligned accumulation regions: q,k | v
                    nc.tensor.matmul(
                        out=ps[:, 0:512], lhsT=lhsT, rhs=w_sb[:, ct, 0:512],
                        start=(ct == 0), stop=(ct == CT - 1),
                    )
                    nc.tensor.matmul(
                        out=ps[:, 512:QKV], lhsT=lhsT, rhs=w_sb[:, ct, 512:QKV],
                        start=(ct == 0), stop=(ct == CT - 1),
                    )
                # drain on DVE fuses the bias add: bq replicated across
                # partitions (zeros for k,v; k bias cancels in softmax,
                # v bias is folded into bout on the host)
                nc.vector.tensor_tensor(
                    out=qt4[:, i, :], in0=ps, in1=bq_sb, op=ADD,
                )
            # rows (sg*4+i)*128 + w of the row-major copy -> quarter sg//8
            qi, sgq = sg // 8, sg % 8
            dstA = qkvQ[qi][sgq * 512 : (sgq + 1) * 512, :].rearrange(
                "(i p) o -> p i o", p=128
            )
            nc.scalar.dma_start(out=dstA, in_=qt4)
            # rows w*128 + (sg*4+i) of the col-major q,k copy
            dstB = qkvB2[:].rearrange("(p i) o -> p i o", i=ST)[
                :, sg * 4 : (sg + 1) * 4, :
            ]
            nc.scalar.dma_start(out=dstB, in_=qt4[:, :, 0:512])

    # ---------- phase 2: axial attention (8-tile chunks) ----------
    with (
        tc.tile_pool(name="a_psS", bufs=1, space="PSUM") as psumS,
        tc.tile_pool(name="a_psZ", bufs=1, space="PSUM") as psumZ,
        tc.tile_pool(name="a_psO", bufs=2, space="PSUM") as psumO,
    ):
        for branch in range(2):  # 0 = row (writes O), 1 = col (adds into O)
            for tg in range(ST // 8):  # chunks of 8 attention tiles
                if branch == 0:
                    rows = qkvQ[tg // 4][(tg % 4) * 1024 : (tg % 4 + 1) * 1024, :]
                else:
                    rows = qkvB2[tg * 1024 : (tg + 1) * 1024, :]
                vt8 = vtpool.tile([128, 8, 256], BF16)
                if branch == 0:
                    nc.gpsimd.dma_start(
                        out=vt8,
                        in_=rows[:, 512:768].rearrange("(i p) o -> p i o", p=128),
                    )
                else:
                    # v for col tiles w = tg*8+i: element (g=h, i, d) lives in
                    # quarter qi at row (h - 32*qi)*128 + w, col 512 + d
                    for qi in range(NQ):
                        src2 = qkvQ[qi][:].rearrange(
                            "(h w) o -> h w o", w=W
                        )[:, tg * 8 : (tg + 1) * 8, 512:768]
                        eng = nc.scalar if qi < 2 else nc.gpsimd
                        eng.dma_start(
                            out=vt8[32 * qi : 32 * (qi + 1), :, :], in_=src2
                        )
                for hp in range(2):  # head pair
                    q8 = qtpool.tile([128, 1024], BF16)
                    nc.sync.dma_start_transpose(
                        out=q8, in_=rows[:, hp * 128 : (hp + 1) * 128]
                    )
                    k8 = ktpool.tile([128, 1024], BF16)
                    nc.sync.dma_start_transpose(
                        out=k8, in_=rows[:, 256 + hp * 128 : 256 + (hp + 1) * 128]
                    )
                    # PV output of both heads stacked into one [128, 512] bank
                    # per half-chunk j; Z likewise hl-stacked per half-chunk
                    # (partition-offset MATMUL writes are HW-safe)
                    psO = [psumO.tile([128, 512], F32, name=f"psO{j}") for j in range(2)]
                    psZs = psumZ.tile([128, 1024], F32, name="psZs", tag="psZ")
                    for hl in range(2):  # head within pair
                        r0, r1 = hl * 64, (hl + 1) * 64
                        psS = psumS.tile([128, 1024], F32)
                        for i in range(8):
                            nc.tensor.matmul(
                                out=psS[:, i * 128 : (i + 1) * 128],
                                lhsT=k8[r0:r1, i * 128 : (i + 1) * 128],
                                rhs=q8[r0:r1, i * 128 : (i + 1) * 128],
                                start=True, stop=True,
                            )
                        pch = ppool.tile([128, 1024], BF16)
                        nc.scalar.activation(
                            out=pch, in_=psS, func=EXP, scale=float(SCALE)
                        )
                        for j in range(2):
                            nc.tensor.matmul(
                                out=psZs[r0:r1, j * 512 : (j + 1) * 512],
                                lhsT=ones_sb[:, 0:64],
                                rhs=pch[:, j * 512 : (j + 1) * 512],
                                start=True, stop=True,
                            )
                        for j in range(2):
                            for i in range(4):
                                ii = j * 4 + i
                                nc.tensor.matmul(
                                    out=psO[j][r0:r1, i * 128 : (i + 1) * 128],
                                    lhsT=vt8[:, ii, hp * 128 + r0 : hp * 128 + r1],
                                    rhs=pch[:, ii * 128 : (ii + 1) * 128],
                                    start=True, stop=True,
                                )
                    rzs = rzpool.tile([128, 1024], F32)
                    nc.vector.reciprocal_approx_fast(out=rzs, in_=psZs)
                    for j in range(2):  # drain both heads at once
                        t0 = tg * 8 + j * 4  # first tile of this half
                        if branch == 0:
                            nc.vector.tensor_tensor(
                                out=O_sb[hp][:, t0 * 128 : t0 * 128 + 512],
                                in0=psO[j],
                                in1=rzs[:, j * 512 : (j + 1) * 512],
                                op=MULT,
                            )
                        else:
                            tmp = tmppool.tile([128, 512], BF16)
                            nc.vector.tensor_tensor(
                                out=tmp, in0=psO[j],
                                in1=rzs[:, j * 512 : (j + 1) * 512],
                                op=MULT,
                            )
                            dst = O_sb[hp][:, :].rearrange(
                                "p (h w) -> p h w", w=W
                            )[:, :, t0 : t0 + 4]
                            nc.gpsimd.tensor_tensor(
                                out=dst,
                                in0=tmp.rearrange("p (w h) -> p h w", w=4),
                                in1=dst, op=ADD,
                            )

    # ---------- phase 3: output projection ----------
    out_r = out.ap().rearrange("(t p) s -> p t s", p=128)
    with tc.tile_pool(name="f_ps", bufs=3, space="PSUM") as psumF:
        for ch in range(S // 512):
            of4 = fpool.tile([128, CT, 512], BF16)
            for ot in range(CT):
                psF = psumF.tile([128, 512], F32)
                for hp in range(2):
                    nc.tensor.matmul(
                        out=psF,
                        lhsT=wout_sb[:, hp, ot * 128 : (ot + 1) * 128],
                        rhs=O_sb[hp][:, ch * 512 : (ch + 1) * 512],
                        start=(hp == 0), stop=(hp == 1),
                    )
                if ot < 2:
                    nc.scalar.activation(
                        out=of4[:, ot, :], in_=psF, func=IDENT,
                        bias=boutv[:, ot : ot + 1], scale=1.0,
                    )
                else:
                    nc.vector.tensor_scalar_add(
                        out=of4[:, ot, :], in0=psF, scalar1=boutv[:, ot : ot + 1]
                    )
            nc.scalar.dma_start(
                out=out_r[:, :, ch * 512 : (ch + 1) * 512], in_=of4
            )


def get_nc():
    global _CACHED_NC
    if _CACHED_NC is None:
        _CACHED_NC = build_nc()
    return _CACHED_NC


def make_in_maps(x, Wqkv, bqkv, Wout, bout):
    """Per-core input dicts: core c = (b, g) with b = c // 2, g = c % 2."""
    in_maps = []
    # v-bias folds into the output bias: out += Wout @ (2 * bv_full)
    bv = bqkv[1024:1536].astype(np.float64)
    bout_folded = (
        bout.astype(np.float64) + 2.0 * (Wout.astype(np.float64) @ bv)
    ).astype(np.float32)
    for c in range(8):
        b, g = c // 2, c % 2
        sel = slice(256 * g, 256 * (g + 1))
        wsel = np.concatenate(
            [Wqkv[sel, :], Wqkv[512 + 256 * g : 512 + 256 * (g + 1), :],
             Wqkv[1024 + 256 * g : 1024 + 256 * (g + 1), :]], axis=0
        )  # [768, 512]
        bqsel = bqkv[sel]  # q bias for this head group [256]
        woutT = np.ascontiguousarray(Wout[:, sel].T)  # [256, 512]
        in_maps.append(
            {
                "x": np.ascontiguousarray(x[b].reshape(C, S)).astype(
                    ml_dtypes.bfloat16
                ),
                "wqkvT": np.ascontiguousarray(wsel.T).astype(ml_dtypes.bfloat16),
                "bq": np.broadcast_to(
                    np.concatenate([bqsel, np.zeros(2 * GC, np.float32)]),
                    (128, QKV),
                ).copy(),
                                "woutT": woutT.astype(ml_dtypes.bfloat16),
                "bout": (
                    np.ascontiguousarray(bout_folded.reshape(CT, 128).T)
                    if g == 0
                    else np.zeros((128, CT), np.float32)
                ),
            }
        )
    return in_maps


def kernel(x, Wqkv, bqkv, Wout, bout):
    x = np.asarray(x, dtype=np.float32)
    Wqkv = np.asarray(Wqkv, dtype=np.float32)
    bqkv = np.asarray(bqkv, dtype=np.float32)
    Wout = np.asarray(Wout, dtype=np.float32)
    bout = np.asarray(bout, dtype=np.float32)

    nc = get_nc()
    in_maps = make_in_maps(x, Wqkv, bqkv, Wout, bout)
    res = run_bass_kernel_spmd(nc, in_maps, core_ids=list(range(8)))
    B = x.shape[0]
    out = np.empty((B, C, H, W), dtype=np.float32)
    for b in range(B):
        acc = res.results[2 * b]["out"].astype(np.float32) + res.results[
            2 * b + 1
        ]["out"].astype(np.float32)
        out[b] = acc.reshape(C, H, W)
    return out


# revision 39
# speedup vs baseline: 1.0128x; 1.0128x over previous
"""AxialAttention TRN2 kernel.

Sharding: 8 cores = 4 batches x 2 head-groups (4 heads each). Each core:
  phase 1: qkv projection (bf16 matmuls, x stationary) -> s-major qkv [S, 768]
           written to DRAM in BOTH row-major (h,w) and col-major (w,h) pixel
           orders; row-major copy split into 4 quarter tiles. Bias handling:
           k-bias cancels in softmax, v-bias is folded into bout on the host
           (sum_u P_u = 1), so only the q-bias is applied on device, as a
           zero-padded K=1 matmul [bq | 0] closing the bank-aligned q,k
           accumulation region (regions narrower than a PSUM bank with
           mismatched chain ends corrupt the accumulation).
  phase 2: row + col axial attention (bf16): scores via k^T q (d contracted),
           exp without max-subtraction (scores bounded), per-query sums Z via
           a ones-matmul stacked per head pair (psZs), 1/Z on DVE
           (reciprocal_approx_fast; the neuronxcc verifier rejects both the
           divide ALU op and TensorTensor with two PSUM operands), drains
           fused with the normalization multiply. Row branch writes O_sb,
           col branch multiplies into a tmp tile and adds into O_sb on the
           Pool engine (strided dst).
  phase 3: output projection (bf16) of row+col sum; bias on group-0 cores
           only (pre-folded with the v-bias term on host); out dram is bf16,
           host upcasts and sums the two head-group partials.

SBUF pools span all phases (no inter-phase SBUF anti-deps); PSUM pools are
phase-scoped. DMA routing: nc.sync carries ONLY transpose DMAs (XBAR
transpose<->copy interleave corrupts data on this HW), nc.scalar the phase-1
streams / col-v quarters 0-1 / out, nc.gpsimd (SWDGE) the row-v stream and
col-v quarters 2-3.
"""

import numpy as np
import ml_dtypes
from contextlib import ExitStack

import concourse.bass as bass
import concourse.bacc as bacc
import concourse.tile as tile
from concourse import mybir
from concourse.bass_utils import run_bass_kernel_spmd

C = 512          # channels
H = 128          # height
W = 128          # width
S = H * W        # 16384 pixels
NH = 8           # total heads
D = 64           # head dim
NHC = 4          # heads per core
GC = NHC * D     # 256 group channels (q or k or v)
QKV = 3 * GC     # 768 projected channels per core
CT = C // 128    # 4 contraction tiles
ST = S // 128    # 128 spatial tiles
NQ = 4           # qkvB quarter tiles
SCALE = 1.0 / np.sqrt(D)

F32 = mybir.dt.float32
BF16 = mybir.dt.bfloat16
EXP = mybir.ActivationFunctionType.Exp
IDENT = mybir.ActivationFunctionType.Identity
ADD = mybir.AluOpType.add
MULT = mybir.AluOpType.mult
DIV = mybir.AluOpType.divide

_CACHED_NC = None


def build_nc(debug_dump=False, reps=1):
    nc = bacc.Bacc()
    x_in = nc.dram_tensor("x", [C, S], BF16, kind="ExternalInput")
    wqkvT = nc.dram_tensor("wqkvT", [C, QKV], BF16, kind="ExternalInput")
    bq = nc.dram_tensor("bq", [128, QKV], F32, kind="ExternalInput")
    woutT = nc.dram_tensor("woutT", [GC, C], BF16, kind="ExternalInput")
    bout = nc.dram_tensor("bout", [128, CT], F32, kind="ExternalInput")
    out = nc.dram_tensor("out", [C, S], BF16, kind="ExternalOutput")
    if debug_dump:
        dbg_qkv = nc.dram_tensor("dbg_qkv", [S, QKV], BF16, kind="ExternalOutput")
        dbg_O = nc.dram_tensor("dbg_O", [2, 128, S], BF16, kind="ExternalOutput")

    with tile.TileContext(nc) as tc, ExitStack() as ctx:
        persist = ctx.enter_context(tc.tile_pool(name="persist", bufs=1))
        dram = ctx.enter_context(tc.tile_pool(name="dram", bufs=1, space="DRAM"))

        # --- persistent tiles ---
        w_sb = persist.tile([128, CT, QKV], BF16, tag="w_sb")
        nc.scalar.dma_start(
            out=w_sb, in_=wqkvT.ap().rearrange("(t p) o -> p t o", p=128)
        )
        bq_sb = persist.tile([128, QKV], F32, tag="bq_sb")
        nc.scalar.dma_start(out=bq_sb, in_=bq.ap())
        wout_sb = persist.tile([128, 2, C], BF16, tag="wout_sb")
        nc.scalar.dma_start(
            out=wout_sb, in_=woutT.ap().rearrange("(t p) o -> p t o", p=128)
        )
        boutv = persist.tile([128, CT], F32, tag="boutv")
        nc.scalar.dma_start(out=boutv, in_=bout.ap())
        ones_sb = persist.tile([128, 128], BF16, tag="ones_sb")
        nc.vector.memset(ones_sb, 1.0)

        O_sb = [
            persist.tile([128, S], BF16, tag=f"O{i}", name=f"O{i}") for i in range(2)
        ]

        # row-major copy in quarters (pixel order s = h*W + w)
        qkvQ = [
            dram.tile([S // NQ, QKV], BF16, tag=f"qkvQ{i}", name=f"qkvQ{i}")
            for i in range(NQ)
        ]
        qkvB2 = dram.tile([S, 512], BF16)   # q,k only; pixel order s' = w*H + h

        for _rep in range(reps):
            build_body(nc, tc, x_in, w_sb, bq_sb, wout_sb, boutv,
                       ones_sb, O_sb, qkvQ, qkvB2, out)

        if debug_dump:
            for qi in range(NQ):
                nc.scalar.dma_start(
                    out=dbg_qkv[qi * (S // NQ) : (qi + 1) * (S // NQ), :],
                    in_=qkvQ[qi][:],
                )
            for i in range(2):
                nc.scalar.dma_start(out=dbg_O.ap()[i], in_=O_sb[i])

    nc.finalize()
    return nc


def build_body(nc, tc, x_in, w_sb, bq_sb, wout_sb, boutv, ones_sb,
               O_sb, qkvQ, qkvB2, out):
    # SBUF pools span all phases (no SBUF anti-deps between phases);
    # PSUM pools stay phase-scoped (8 banks cannot hold all phases at once).
    sbctx = ExitStack()
    with sbctx:
        xpool = sbctx.enter_context(tc.tile_pool(name="p1x", bufs=4))
        opool = sbctx.enter_context(tc.tile_pool(name="p1o", bufs=3))
        qtpool = sbctx.enter_context(tc.tile_pool(name="a_qt", bufs=4))
        ktpool = sbctx.enter_context(tc.tile_pool(name="a_kt", bufs=4))
        vtpool = sbctx.enter_context(tc.tile_pool(name="a_vt", bufs=2))
        ppool = sbctx.enter_context(tc.tile_pool(name="a_p", bufs=3))
        tmppool = sbctx.enter_context(tc.tile_pool(name="a_tmp", bufs=3))
        rzpool = sbctx.enter_context(tc.tile_pool(name="a_rz", bufs=2))
        fpool = sbctx.enter_context(tc.tile_pool(name="f_o", bufs=3))
        build_phases(nc, tc, x_in, w_sb, bq_sb, wout_sb, boutv,
                     ones_sb, O_sb, qkvQ, qkvB2, out, xpool, opool, qtpool,
                     ktpool, vtpool, ppool, tmppool, rzpool, fpool)


def build_phases(nc, tc, x_in, w_sb, bq_sb, wout_sb, boutv, ones_sb,
                 O_sb, qkvQ, qkvB2, out, xpool, opool, qtpool, ktpool, vtpool,
                 ppool, tmppool, rzpool, fpool):
    # ---------- phase 1: qkv projection (x stationary, s-major out) ----------
    x_r = x_in.ap().rearrange("(t p) s -> p t s", p=128)
    with tc.tile_pool(name="p1ps", bufs=3, space="PSUM") as pspool:
        for sg in range(ST // 4):  # groups of 4 s-tiles (one h-quad)
            if sg % 2 == 0:
                xg = xpool.tile([128, CT, 1024], BF16)
                nc.scalar.dma_start(
                    out=xg, in_=x_r[:, :, sg * 512 : sg * 512 + 1024]
                )
            qt4 = opool.tile([128, 4, QKV], BF16)
            for i in range(4):
                ps = pspool.tile([128, QKV], F32)
                for ct in range(CT):
                    lhsT = xg[
                        :, ct,
                        (sg % 2) * 512 + i * 128 : (sg % 2) * 512 + (i + 1) * 128,
                    ]
                    # two bank-aligned accumulation regions: q,k | v
                    nc.tensor.matmul(
                        out=ps[:, 0:512], lhsT=lhsT, rhs=w_sb[:, ct, 0:512],
                        start=(ct == 0), stop=(ct == CT - 1),
                    )
                    nc.tensor.matmul(
                        out=ps[:, 512:QKV], lhsT=lhsT, rhs=w_sb[:, ct, 512:QKV],
                        start=(ct == 0), stop=(ct == CT - 1),
                    )
                # drain on DVE fuses the bias add: bq replicated across
                # partitions (zeros for k,v; k bias cancels in softmax,
                # v bias is folded into bout on the host)
                nc.vector.tensor_tensor(
                    out=qt4[:, i, :], in0=ps, in1=bq_sb, op=ADD,
                )
            # rows (sg*4+i)*128 + w of the row-major copy -> quarter sg//8
            qi, sgq = sg // 8, sg % 8
            dstA = qkvQ[qi][sgq * 512 : (sgq + 1) * 512, :].rearrange(
                "(i p) o -> p i o", p=128
            )
            nc.scalar.dma_start(out=dstA, in_=qt4)
            # rows w*128 + (sg*4+i) of the col-major q,k copy
            dstB = qkvB2[:].rearrange("(p i) o -> p i o", i=ST)[
                :, sg * 4 : (sg + 1) * 4, :
            ]
            nc.scalar.dma_start(out=dstB, in_=qt4[:, :, 0:512])

    # ---------- phase 2: axial attention (8-tile chunks) ----------
    with (
        tc.tile_pool(name="a_psS", bufs=1, space="PSUM") as psumS,
        tc.tile_pool(name="a_psZ", bufs=1, space="PSUM") as psumZ,
        tc.tile_pool(name="a_psO", bufs=2, space="PSUM") as psumO,
    ):
        for branch in range(2):  # 0 = row (writes O), 1 = col (adds into O)
            for tg in range(ST // 8):  # chunks of 8 attention tiles
                if branch == 0:
                    rows = qkvQ[tg // 4][(tg % 4) * 1024 : (tg % 4 + 1) * 1024, :]
                else:
                    rows = qkvB2[tg * 1024 : (tg + 1) * 1024, :]
                vt8 = vtpool.tile([128, 8, 256], BF16)
                if branch == 0:
                    nc.gpsimd.dma_start(
                        out=vt8,
                        in_=rows[:, 512:768].rearrange("(i p) o -> p i o", p=128),
                    )
                else:
                    # v for col tiles w = tg*8+i: element (g=h, i, d) lives in
                    # quarter qi at row (h - 32*qi)*128 + w, col 512 + d
                    for qi in range(NQ):
                        src2 = qkvQ[qi][:].rearrange(
                            "(h w) o -> h w o", w=W
                        )[:, tg * 8 : (tg + 1) * 8, 512:768]
                        eng = nc.scalar if qi < 2 else nc.gpsimd
                        eng.dma_start(
                            out=vt8[32 * qi : 32 * (qi + 1), :, :], in_=src2
                        )
                for hp in range(2):  # head pair
                    q8 = qtpool.tile([128, 1024], BF16)
                    nc.sync.dma_start_transpose(
                        out=q8, in_=rows[:, hp * 128 : (hp + 1) * 128]
                    )
                    k8 = ktpool.tile([128, 1024], BF16)
                    nc.sync.dma_start_transpose(
                        out=k8, in_=rows[:, 256 + hp * 128 : 256 + (hp + 1) * 128]
                    )
                    # PV output of both heads stacked into one [128, 512] bank
                    # per half-chunk j; Z likewise hl-stacked per half-chunk
                    # (partition-offset MATMUL writes are HW-safe)
                    psO = [psumO.tile([128, 512], F32, name=f"psO{j}") for j in range(2)]
                    psZs = psumZ.tile([128, 1024], F32, name="psZs", tag="psZ")
                    for hl in range(2):  # head within pair
                        r0, r1 = hl * 64, (hl + 1) * 64
                        psS = psumS.tile([128, 1024], F32)
                        for i in range(8):
                            nc.tensor.matmul(
                                out=psS[:, i * 128 : (i + 1) * 128],
                                lhsT=k8[r0:r1, i * 128 : (i + 1) * 128],
                                rhs=q8[r0:r1, i * 128 : (i + 1) * 128],
                                start=True, stop=True,
                            )
                        pch = ppool.tile([128, 1024], BF16)
                        nc.scalar.activation(
                            out=pch, in_=psS, func=EXP, scale=float(SCALE)
                        )
                        for j in range(2):
                            nc.tensor.matmul(
                                out=psZs[r0:r1, j * 512 : (j + 1) * 512],
                                lhsT=ones_sb[:, 0:64],
                                rhs=pch[:, j * 512 : (j + 1) * 512],
                                start=True, stop=True,
                            )
                        for j in range(2):
                            for i in range(4):
                                ii = j * 4 + i
                                nc.tensor.matmul(
                                    out=psO[j][r0:r1, i * 128 : (i + 1) * 128],
                                    lhsT=vt8[:, ii, hp * 128 + r0 : hp * 128 + r1],
                                    rhs=pch[:, ii * 128 : (ii + 1) * 128],
                                    start=True, stop=True,
                                )
                    rzs = rzpool.tile([128, 1024], F32)
                    nc.vector.reciprocal_approx_fast(out=rzs, in_=psZs)
                    for j in range(2):  # drain both heads at once
                        t0 = tg * 8 + j * 4  # first tile of this half
                        if branch == 0:
                            nc.vector.tensor_tensor(
                                out=O_sb[hp][:, t0 * 128 : t0 * 128 + 512],
                                in0=psO[j],
                                in1=rzs[:, j * 512 : (j + 1) * 512],
                                op=MULT,
                            )
                        else:
                            tmp = tmppool.tile([128, 512], BF16)
                            nc.vector.tensor_tensor(
                                out=tmp, in0=psO[j],
                                in1=rzs[:, j * 512 : (j + 1) * 512],
                                op=MULT,
                            )
                            dst = O_sb[hp][:, :].rearrange(
                                "p (h w) -> p h w", w=W
                            )[:, :, t0 : t0 + 4]
                            nc.gpsimd.tensor_tensor(
                                out=dst,
                                in0=tmp.rearrange("p (w h) -> p h w", w=4),
                                in1=dst, op=ADD,
                            )

    # ---------- phase 3: output projection ----------
    out_r = out.ap().rearrange("(t p) s -> p t s", p=128)
    with tc.tile_pool(name="f_ps", bufs=6, space="PSUM") as psumF:
        for ch in range(S // 512):
            of4 = fpool.tile([128, CT, 512], BF16)
            for ot in range(CT):
                psF = psumF.tile([128, 512], F32)
                for hp in range(2):
                    nc.tensor.matmul(
                        out=psF,
                        lhsT=wout_sb[:, hp, ot * 128 : (ot + 1) * 128],
                        rhs=O_sb[hp][:, ch * 512 : (ch + 1) * 512],
                        start=(hp == 0), stop=(hp == 1),
                    )
                if ot < 2:
                    nc.scalar.activation(
                        out=of4[:, ot, :], in_=psF, func=IDENT,
                        bias=boutv[:, ot : ot + 1], scale=1.0,
                    )
                else:
                    nc.vector.tensor_scalar_add(
                        out=of4[:, ot, :], in0=psF, scalar1=boutv[:, ot : ot + 1]
                    )
            nc.scalar.dma_start(
                out=out_r[:, :, ch * 512 : (ch + 1) * 512], in_=of4
            )


def get_nc():
    global _CACHED_NC
    if _CACHED_NC is None:
        _CACHED_NC = build_nc()
    return _CACHED_NC


def make_in_maps(x, Wqkv, bqkv, Wout, bout):
    """Per-core input dicts: core c = (b, g) with b = c // 2, g = c % 2."""
    in_maps = []
    # v-bias folds into the output bias: out += Wout @ (2 * bv_full)
    bv = bqkv[1024:1536].astype(np.float64)
    bout_folded = (
        bout.astype(np.float64) + 2.0 * (Wout.astype(np.float64) @ bv)
    ).astype(np.float32)
    for c in range(8):
        b, g = c // 2, c % 2
        sel = slice(256 * g, 256 * (g + 1))
        wsel = np.concatenate(
            [Wqkv[sel, :], Wqkv[512 + 256 * g : 512 + 256 * (g + 1), :],
             Wqkv[1024 + 256 * g : 1024 + 256 * (g + 1), :]], axis=0
        )  # [768, 512]
        bqsel = bqkv[sel]  # q bias for this head group [256]
        woutT = np.ascontiguousarray(Wout[:, sel].T)  # [256, 512]
        in_maps.append(
            {
                "x": np.ascontiguousarray(x[b].reshape(C, S)).astype(
                    ml_dtypes.bfloat16
                ),
                "wqkvT": np.ascontiguousarray(wsel.T).astype(ml_dtypes.bfloat16),
                "bq": np.broadcast_to(
                    np.concatenate([bqsel, np.zeros(2 * GC, np.float32)]),
                    (128, QKV),
                ).copy(),
                                "woutT": woutT.astype(ml_dtypes.bfloat16),
                "bout": (
                    np.ascontiguousarray(bout_folded.reshape(CT, 128).T)
                    if g == 0
                    else np.zeros((128, CT), np.float32)
                ),
            }
        )
    return in_maps


def kernel(x, Wqkv, bqkv, Wout, bout):
    x = np.asarray(x, dtype=np.float32)
    Wqkv = np.asarray(Wqkv, dtype=np.float32)
    bqkv = np.asarray(bqkv, dtype=np.float32)
    Wout = np.asarray(Wout, dtype=np.float32)
    bout = np.asarray(bout, dtype=np.float32)

    nc = get_nc()
    in_maps = make_in_maps(x, Wqkv, bqkv, Wout, bout)
    res = run_bass_kernel_spmd(nc, in_maps, core_ids=list(range(8)))
    B = x.shape[0]
    out = np.empty((B, C, H, W), dtype=np.float32)
    for b in range(B):
        acc = res.results[2 * b]["out"].astype(np.float32) + res.results[
            2 * b + 1
        ]["out"].astype(np.float32)
        out[b] = acc.reshape(C, H, W)
    return out
